# revision 13
# baseline (speedup 1.0000x reference)
"""Multi-head attention (B=2, T=2048, D=1024, H=16, no causal mask) on 8 trn2
NeuronCores.

Sharding: data-parallel over (batch, token-block).  Core c owns batch
b = c // 4 and tokens [tb*512, (tb+1)*512), tb = c % 4.  Each core computes
Q/K/V only for its OWN 512 tokens; the K and V needed for full attention over
all 2048 keys are exchanged with the 3 sibling cores of the same batch via
4-rank AllGathers (replica groups [[0..3],[4..7]]).  Since the program is
SPMD (identical on every core), kt_sb / v_sb are filled entirely from the
AllGather outputs (rank order == absolute token order on every core), keeping
all addresses position-independent.  Each AllGather is split in two halves
(K by head-pair group, V by head group) so attention on early pairs can start
while the later half is still in flight.

Engine assignment:
  PE:     transposes, Q/K/V projections, logits (two heads of a pair run as
          concurrent row-tiles at partitions 0-63/64-127), PV, o-proj.
  ACT:    exp ONLY (it is the bottleneck engine: 128 x [128,1024] calls).
  DVE:    PSUM->SBUF copies, reciprocal_approx_fast, normalize muls.
  GpSimd: collective triggers, o-proj PSUM->SBUF copies.

Per-core plan:
  1. PE-transpose own x block into xqt [1024, 512].
  2. KT own [1024, 512] -> stage -> AllGather (2 halves) -> kt_sb [128, 8*2048]
     V own [512, 1024]  -> stage -> AllGather (2 halves) -> v_sb 65-wide head
     slots (ones column at dh=64 -> PV also yields the softmax sum).
     QT own -> qt_sb.
  3. Attention per head pair p (heads 2p/2p+1 at partitions 0-63/64-127):
       logitsT[k,q] = KT_h^T @ QT_h   (concurrent row-tiles, PSUM [128, 1024]
                                       per 2-half group)
       PT = exp(0.125 * logitsT)      (ScalarE, PSUM -> fp16 SBUF)
       outT_h[dh,q], s[q] = [V_h | 1]^T @ PT   (PSUM [65, 512], accum 16 kc)
     normalize: outT_h *= (1/s) via reciprocal_approx_fast + DMA broadcast.
  4. y[q,:] = outT^T @ Wo (fp16), accumulate 8 row chunks.
"""

import numpy as np

import concourse.bacc as bacc
import concourse.mybir as mybir
import concourse.tile as tile
from concourse.masks import make_identity

F32 = mybir.dt.float32
F16 = mybir.dt.float16

B, T, D, H = 2, 2048, 1024, 16
DH = D // H  # 64
TQ = 512     # tokens owned per core
N_CORES = 8
P = 128
KD = D // P        # 8 contraction chunks over D
NT = T // P        # 16 key-token chunks
NTB = T // TQ      # 4 token blocks (= ranks per replica group)
NPAIR = H // 2     # 8 head pairs
VW = DH + 1        # 65: V head slot width incl. ones column
VWP = 80           # padded slot stride: 160B = 5*32B keeps DMA writes aligned
NQ = TQ // P       # 4 query-token chunks
GRP = 2            # logit halves per psum tile / exp call
EXPF = mybir.ActivationFunctionType.Exp
RG = [[0, 1, 2, 3], [4, 5, 6, 7]]


DEBUG_DUMPS = False


def build_nc():
    nc = bacc.Bacc("TRN2", target_bir_lowering=False, debug=False,
                   num_devices=N_CORES)
    xq = nc.dram_tensor("xq", [TQ, D], F16, kind="ExternalInput").ap()
    wqkv = nc.dram_tensor("wqkv", [D, 3 * D], F16, kind="ExternalInput").ap()
    wo = nc.dram_tensor("wo", [D, D], F16, kind="ExternalInput").ap()
    y = nc.dram_tensor("y", [TQ, D], F32, kind="ExternalOutput").ap()
    if DEBUG_DUMPS:
        dbg_kt = nc.dram_tensor("dbg_kt", [P, KD * T], F16,
                                kind="ExternalOutput").ap()
        dbg_v = nc.dram_tensor("dbg_v", [P, NT * H * VWP], F16,
                               kind="ExternalOutput").ap()
        dbg_qt = nc.dram_tensor("dbg_qt", [P, NPAIR * TQ], F16,
                                kind="ExternalOutput").ap()
        dbg_ot = nc.dram_tensor("dbg_ot", [P, NPAIR * TQ], F16,
                                kind="ExternalOutput").ap()
        dbg_v2 = nc.dram_tensor("dbg_v2", [P, NT * H * VWP], F16,
                                kind="ExternalOutput").ap()

    with tile.TileContext(nc) as tc:
      with tc.tile_pool(name="persist", bufs=1) as persist:
        v_sb = persist.tile([P, NT * H * VWP], F16)   # 40 KB/part
        qt_sb = persist.tile([P, NPAIR * TQ], F16)    # 8 KB/part
        kt_sb = persist.tile([P, KD * T], F16)        # 32 KB/part
        ident = persist.tile([P, P], F16)
        make_identity(nc, ident)
        # ones columns in every (tok-chunk, head) V slot
        onec = persist.tile([P, 1], F16)
        nc.vector.memset(onec[:], 1.0)
        nc.vector.tensor_copy(
            v_sb.rearrange("p (b c) -> p b c", c=VWP)[:, :, DH:DH + 1],
            onec.unsqueeze(1).broadcast_to((P, NT * H, 1)))

        # wqkv viewed as [p, ko, col]: one DMA per weight column strip
        wq3 = wqkv.rearrange("(ko p) c -> p ko c", p=P)

        with tc.tile_pool(name="dram", bufs=1, space="DRAM") as dram:
            # K staged dims-major: [dims 512, tok 512] per half
            cc_in_k = [dram.tile([4 * P, TQ], F16, name=f"cc_in_k{i}")
                       for i in range(2)]
            cc_out_k = [dram.tile([NTB * 4 * P, TQ], F16, name=f"cc_out_k{i}")
                        for i in range(2)]
            # V staged natural: [tok 512, dims 512] per half (heads 8nh..)
            cc_in_v = [dram.tile([TQ, TQ], F16, name=f"cc_in_v{i}")
                       for i in range(2)]
            cc_out_v = [dram.tile([NTB * TQ, TQ], F16, name=f"cc_out_v{i}")
                        for i in range(2)]

            with (
                tc.tile_pool(name="xtp", bufs=1) as xtp,
                tc.tile_pool(name="wp", bufs=1) as wp,
            ):
                xqt = xtp.tile([P, KD * TQ], F16)   # 8 KB/part
                ktloc = xtp.tile([P, KD * TQ], F16)  # own KT staging
                vloc = xtp.tile([P, NQ * D], F16)    # own V staging (natural)

                with (
                    tc.tile_pool(name="xin", bufs=2) as xinp,
                    tc.tile_pool(name="trps", bufs=2, space="PSUM") as trps,
                    tc.tile_pool(name="pjps", bufs=5, space="PSUM") as pjps,
                ):
                    # A: PE-transpose own x -> xqt
                    for tci in range(NQ):
                        xin = xinp.tile([P, D], F16, tag="xin")
                        nc.sync.dma_start(xin[:], xq[tci * P:(tci + 1) * P, :])
                        ps = trps.tile([P, KD * P], F16, tag="tr")
                        for kd in range(KD):
                            nc.tensor.transpose(
                                ps[:, kd * P:(kd + 1) * P],
                                xin[:, kd * P:(kd + 1) * P], ident[:])
                        nc.vector.tensor_copy(
                            xqt.rearrange("p (k c) -> p k c", c=TQ)
                               [:, :, tci * P:(tci + 1) * P],
                            ps.rearrange("p (k c) -> p k c", c=P))

                    # B: own KT (dims-major), staged + AllGathered in halves
                    for half in range(2):
                        for dhalf in range(KD // 2):
                            do = half * (KD // 2) + dhalf
                            wt = wp.tile([P, KD * P], F16, tag="wk", bufs=2)
                            nc.sync.dma_start(
                                wt.rearrange("p (ko c) -> p ko c", c=P),
                                wq3[:, :, D + do * P: D + (do + 1) * P])
                            pk = pjps.tile([P, TQ], F32, tag="pj")
                            for kd in range(KD):
                                nc.tensor.matmul(
                                    pk[:], wt[:, kd * P:(kd + 1) * P],
                                    xqt[:, kd * TQ:(kd + 1) * TQ],
                                    start=(kd == 0), stop=(kd == KD - 1))
                            nc.vector.tensor_copy(
                                ktloc[:, do * TQ:(do + 1) * TQ], pk[:])
                        # stage own KT half: [p, dhalf, t] -> dram [dhalf*P+p, t]
                        nc.sync.dma_start(
                            cc_in_k[half].rearrange("(dd p) t -> p dd t", p=P),
                            ktloc.rearrange("p (dd t) -> p dd t", t=TQ)
                                 [:, half * 4:(half + 1) * 4, :])
                        nc.gpsimd.collective_compute(
                            "AllGather", mybir.AluOpType.bypass,
                            replica_groups=RG,
                            ins=[cc_in_k[half][:].opt()],
                            outs=[cc_out_k[half][:].opt()])

                    # C: own V (natural), staged + AllGathered in halves
                    for nh in range(2):
                        wvt = wp.tile([P, KD * TQ], F16, tag="wv", bufs=2)
                        nc.sync.dma_start(
                            wvt.rearrange("p (ko c) -> p ko c", c=TQ),
                            wq3[:, :, 2 * D + nh * TQ: 2 * D + (nh + 1) * TQ])
                        for tci in range(NQ):
                            pv = pjps.tile([P, TQ], F32, tag="pj")
                            for kd in range(KD):
                                nc.tensor.matmul(
                                    pv[:],
                                    xqt[:, kd * TQ + tci * P:
                                        kd * TQ + (tci + 1) * P],
                                    wvt[:, kd * TQ:(kd + 1) * TQ],
                                    start=(kd == 0), stop=(kd == KD - 1))
                            nc.vector.tensor_copy(
                                vloc[:, tci * D + nh * TQ:
                                     tci * D + (nh + 1) * TQ], pv[:])
                        # stage own V half: [p, tci, d] -> dram [tci*P+p, d]
                        nc.sync.dma_start(
                            cc_in_v[nh].rearrange("(tt p) d -> p tt d", p=P),
                            vloc.rearrange("p (tt d) -> p tt d", d=D)
                                [:, :, nh * TQ:(nh + 1) * TQ])
                        nc.gpsimd.collective_compute(
                            "AllGather", mybir.AluOpType.bypass,
                            replica_groups=RG,
                            ins=[cc_in_v[nh][:].opt()],
                            outs=[cc_out_v[nh][:].opt()])

                    # D: QT own -> qt_sb
                    for do in range(KD):
                        wt = wp.tile([P, KD * P], F16, tag="wk", bufs=2)
                        nc.sync.dma_start(
                            wt.rearrange("p (ko c) -> p ko c", c=P),
                            wq3[:, :, do * P:(do + 1) * P])
                        pq = pjps.tile([P, TQ], F32, tag="pj")
                        for kd in range(KD):
                            nc.tensor.matmul(
                                pq[:], wt[:, kd * P:(kd + 1) * P],
                                xqt[:, kd * TQ:(kd + 1) * TQ],
                                start=(kd == 0), stop=(kd == KD - 1))
                        nc.vector.tensor_copy(
                            qt_sb[:, do * TQ:(do + 1) * TQ], pq[:])

                # fill kt_sb from AllGather halves
                # cc_out_k[half][(r*4 + dd)*P + p, t] -> kt_sb[p, do*T + r*TQ + t]
                for half in range(2):
                    for r in range(NTB):
                        nc.sync.dma_start(
                            kt_sb.rearrange("p (dd r t) -> p dd r t",
                                            r=NTB, t=TQ)
                                 [:, half * 4:(half + 1) * 4, r, :],
                            cc_out_k[half].rearrange(
                                "(r dd p) t -> p r dd t", p=P, dd=4)[:, r])
                # fill v_sb head slots from AllGather halves
                # cc_out_v[nh][(r*4 + tci)*P + p, h*DH + c]
                #   -> v_sb[p, (r*4+tci)*(H*VW) + (nh*8+h)*VW + c]
                for nh in range(2):
                    for tt in range(NT):
                        nc.sync.dma_start(
                            v_sb.rearrange("p (tt h c) -> p tt h c",
                                           h=H, c=VWP)
                                [:, tt, nh * 8:(nh + 1) * 8, 0:DH],
                            cc_out_v[nh].rearrange(
                                "(tt p) (h c) -> p tt h c",
                                p=P, c=DH)[:, tt, :, :])

            if DEBUG_DUMPS:
                nc.sync.dma_start(dbg_v2[:], v_sb[:])

            # ---------- attention region ---------------------------------
            with (
                tc.tile_pool(name="otp", bufs=1) as otp,
                tc.tile_pool(name="ptp", bufs=8) as ptp,
                tc.tile_pool(name="rcp", bufs=2) as rcp,
                tc.tile_pool(name="rbp", bufs=3) as rbp,
                tc.tile_pool(name="wop", bufs=16) as wop,
            ):
                ot_sb = otp.tile([P, NPAIR * TQ], F16)      # 8 KB/part
                # preload Wo so the o-proj phase never waits on DMA
                wot = {}
                for ph in range(NPAIR):
                    for nh in range(2):
                        wot[ph, nh] = wop.tile([P, TQ], F16, tag="wo",
                                               name=f"wo_{ph}_{nh}")
                        nc.sync.dma_start(
                            wot[ph, nh][:],
                            wo[ph * P:(ph + 1) * P, nh * TQ:(nh + 1) * TQ])

                with (
                    tc.tile_pool(name="lgps", bufs=2, space="PSUM") as lgps,
                    tc.tile_pool(name="pvps", bufs=4, space="PSUM") as pvps,
                ):
                    for p in range(NPAIR):
                        kt = kt_sb[:, p * T:(p + 1) * T]
                        qh = (qt_sb[0:DH, p * TQ:(p + 1) * TQ],
                              qt_sb[DH:P, p * TQ:(p + 1) * TQ])
                        pva = pvps.tile([VW, TQ], F32, tag="pv")
                        pvb = pvps.tile([VW, TQ], F32, tag="pv")
                        halves = [(kc, hh) for kc in range(NT)
                                  for hh in (0, 1)]
                        groups = [halves[i:i + GRP]
                                  for i in range(0, len(halves), GRP)]
                        loc = {}
                        emitted = set()

                        def emit_pv_ready(done_through, p=p, pva=pva, pvb=pvb,
                                          loc=loc, emitted=emitted):
                            for kc in range(NT):
                                if kc in emitted:
                                    continue
                                if ((kc, 0) not in done_through
                                        or (kc, 1) not in done_through):
                                    return
                                emitted.add(kc)
                                for hh, pv_ in ((0, pva), (1, pvb)):
                                    h = 2 * p + hh
                                    va = v_sb[:, kc * (H * VWP) + h * VWP:
                                              kc * (H * VWP) + h * VWP + VW]
                                    pt_, j = loc[kc, hh]
                                    nc.tensor.matmul(
                                        pv_[:], va,
                                        pt_[:, j * TQ:(j + 1) * TQ],
                                        start=(kc == 0),
                                        stop=(kc == NT - 1))

                        done = set()
                        prev_done = set()
                        for gi, grp in enumerate(groups):
                            emit_pv_ready(prev_done)
                            n = len(grp)
                            lg = lgps.tile([P, GRP * TQ], F32, tag="lg")
                            for j, (kc, hh) in enumerate(grp):
                                nc.tensor.matmul(
                                    lg[:, j * TQ:(j + 1) * TQ],
                                    kt[hh * DH:(hh + 1) * DH,
                                       kc * P:(kc + 1) * P],
                                    qh[hh], start=True, stop=True)
                            pt_ = ptp.tile([P, GRP * TQ], F16, tag="pt")
                            nc.scalar.activation(pt_[:, 0:n * TQ],
                                                 lg[:, 0:n * TQ],
                                                 EXPF, scale=0.125)
                            for j, half in enumerate(grp):
                                loc[half] = (pt_, j)
                            prev_done = set(done)
                            done.update(grp)
                        emit_pv_ready(done)

                        # normalize: outT_h[dh, q] *= 1 / s[q].  Copy psum out
                        # first so the PV banks free fast for the next pair.
                        pvs_t = {}
                        for hi, pv_ in ((0, pva), (1, pvb)):
                            pvs = rcp.tile([VW, TQ], F32, tag="pvs")
                            nc.vector.tensor_copy(pvs[:], pv_[:])
                            pvs_t[hi] = pvs
                        for hi in (0, 1):
                            pvs = pvs_t[hi]
                            rc = rcp.tile([P, TQ], F32, tag="rc")
                            nc.vector.reciprocal(
                                rc[DH:DH + 1, :], pvs[DH:DH + 1, :])
                            rb = rbp.tile([P, TQ], F32, tag="rb")
                            nc.sync.dma_start(
                                rb[0:DH, :],
                                rc[DH:DH + 1, :].unsqueeze(1)
                                  .broadcast_to((1, DH, TQ)))
                            if hi == 0:
                                nc.vector.tensor_mul(
                                    ot_sb[0:DH, p * TQ:(p + 1) * TQ],
                                    pvs[0:DH, :], rb[0:DH, :])
                            else:
                                # head b lands at partitions 64-127 of ot_sb;
                                # DVE cannot shift partitions, so normalize
                                # into a staging tile then DMA-shift.
                                sh = rbp.tile([P, TQ], F16, tag="sh")
                                nc.vector.tensor_mul(
                                    sh[0:DH, :], pvs[0:DH, :], rb[0:DH, :])
                                nc.sync.dma_start(
                                    ot_sb[DH:P, p * TQ:(p + 1) * TQ],
                                    sh[0:DH, :])

                if DEBUG_DUMPS:
                    nc.sync.dma_start(dbg_kt[:], kt_sb[:])
                    nc.sync.dma_start(dbg_v[:], v_sb[:])
                    nc.sync.dma_start(dbg_qt[:], qt_sb[:])
                    nc.sync.dma_start(dbg_ot[:], ot_sb[:])

                # F: y = outT^T @ Wo (ph-outer reuses each stationary twice)
                with tc.tile_pool(name="fps", bufs=4, space="PSUM") as fps:
                  for qc in range(NQ):
                    pys = [fps.tile([P, TQ], F32, tag="f", name=f"py{qc}_{_n}")
                           for _n in range(2)]
                    for ph in range(NPAIR):
                        for nh in range(2):
                            nc.tensor.matmul(
                                pys[nh][:],
                                ot_sb[:, ph * TQ + qc * P:
                                      ph * TQ + (qc + 1) * P],
                                wot[ph, nh][:],
                                start=(ph == 0), stop=(ph == NPAIR - 1))
                    for nh in range(2):
                        ys = rbp.tile([P, TQ], F32, tag="rb")
                        nc.scalar.copy(ys[:], pys[nh][:])
                        nc.sync.dma_start(
                            y[qc * P:(qc + 1) * P, nh * TQ:(nh + 1) * TQ],
                            ys[:])
    nc.compile()
    return nc


_NC_CACHE = None


def _get_nc():
    global _NC_CACHE
    if _NC_CACHE is None:
        _NC_CACHE = build_nc()
    return _NC_CACHE


def _shard_inputs(x, Wqkv, Wo):
    x16 = np.asarray(x, dtype=np.float32).astype(np.float16)
    w16 = np.ascontiguousarray(
        np.asarray(Wqkv, dtype=np.float32).astype(np.float16))
    wo16 = np.ascontiguousarray(
        np.asarray(Wo, dtype=np.float32).astype(np.float16))
    in_maps = []
    for c in range(N_CORES):
        b, tb = c // NTB, c % NTB
        in_maps.append({
            "xq": np.ascontiguousarray(x16[b, tb * TQ:(tb + 1) * TQ, :]),
            "wqkv": w16,
            "wo": wo16,
        })
    return in_maps


def kernel(x, Wqkv, Wo):
    from concourse.bass_utils import run_bass_kernel_spmd

    nc = _get_nc()
    in_maps = _shard_inputs(x, Wqkv, Wo)
    res = run_bass_kernel_spmd(nc, in_maps, core_ids=list(range(N_CORES)))
    out = np.empty((B, T, D), dtype=np.float32)
    for c in range(N_CORES):
        b, tb = c // NTB, c % NTB
        out[b, tb * TQ:(tb + 1) * TQ, :] = res.results[c]["y"]
    return out


# revision 14
# speedup vs baseline: 1.2560x; 1.2560x over previous
"""Multi-head attention (B=2, T=2048, D=1024, H=16, no causal mask) on 8 trn2
NeuronCores.

Sharding: pure data-parallel over (batch, query-token-block).  Core c handles
batch b = c // 4 and query rows [tb*512, (tb+1)*512) with tb = c % 4.  Each
core redundantly computes K and V for its whole batch; collectives were
measured at 40-66us each (serialized) in this runtime, far more than the
redundant compute they would save, so no collectives are used.

Precision: fp16 throughout with fp32 PSUM accumulation (~7e-4 final rel err).

Engine discipline (the attention region is ACT-bound at ~18.3us/pair of exp):
  ACT (scalar): exp + the B-chunk KT copies + o-proj copies (ACT has ~8us/pair
                of slack; putting B copies here keeps them off Vector so the
                normalize reciprocal never blocks a PSUM-bank free that the
                in-order PE queue is waiting on).
  DVE (vector): pre-region PSUM->SBUF copies, pvs copies (emitted FIRST in the
                normalize chain so PV banks free immediately), reciprocal,
                normalize muls.
  PE:           transposes, QKV projections, logits (two heads of a pair run
                as concurrent row-tiles at partitions 0-63/64-127), PV, o-proj.

Per-core plan:
  1. PE-transpose X[b] into XT (1024x2048 SBUF) and the query slice into XqT.
  2. QT[do,:] = Wq[:,do]^T @ XqT       (QT:  [1024, 512]  fp16 SBUF)
     KT[do,:] = Wk[:,do]^T @ XT        (KT:  [1024, 2048] fp16 SBUF)
     V [tc,:] = XT[:,tc]^T @ Wv        (V:   [2048, 1024] fp16 SBUF, 65-wide
                                        head slots with a ones column ->
                                        PV also yields the softmax sum)
  3. Attention per head pair p (heads 2p, 2p+1 at partitions 0-63 / 64-127
     of KT/QT row chunk p):
       logitsT[k,q] = KT_h[:,kc]^T @ QT_h      (PSUM [128, 512] fp32)
       PT = exp(0.125 * logitsT)               (ScalarE, PSUM -> fp16 SBUF)
       outT_h[dh,q], s[q] = [V_h | 1]^T @ PT   (PSUM [65, 512], accum 16 kc)
     normalize: outT_h *= (1/s) broadcast across partitions via DMA.
     KT chunks for do=1..7 are spread through the ACT-bound attention as PE
     filler.
  4. y[q,:] = outT^T @ Wo (fp16), accumulate 8 row chunks.
"""

import numpy as np

import concourse.bacc as bacc
import concourse.mybir as mybir
import concourse.tile as tile
from concourse.masks import make_identity

F32 = mybir.dt.float32
F16 = mybir.dt.float16

B, T, D, H = 2, 2048, 1024, 16
DH = D // H  # 64
TQ = 512     # query tokens per core
N_CORES = 8
P = 128
KD = D // P        # 8 contraction chunks over D
NT = T // P        # 16 key-token chunks
NTB = T // TQ      # 4 token blocks
NPAIR = H // 2     # 8 head pairs
VW = DH + 1        # 65: V head slot width incl. ones column
NQ = TQ // P       # 4 query-token chunks
EXPF = mybir.ActivationFunctionType.Exp


def build_nc():
    nc = bacc.Bacc("TRN2", target_bir_lowering=False, debug=False,
                   num_devices=N_CORES)
    xb = nc.dram_tensor("xb", [T, D], F16, kind="ExternalInput").ap()
    xq = nc.dram_tensor("xq", [TQ, D], F16, kind="ExternalInput").ap()
    wqkv = nc.dram_tensor("wqkv", [D, 3 * D], F16, kind="ExternalInput").ap()
    wo = nc.dram_tensor("wo", [D, D], F16, kind="ExternalInput").ap()
    y = nc.dram_tensor("y", [TQ, D], F32, kind="ExternalOutput").ap()

    with tile.TileContext(nc) as tc:
      with tc.tile_pool(name="persist", bufs=1) as persist:
        v_sb = persist.tile([P, NT * H * VW], F16)    # 32.5 KB/part
        qt_sb = persist.tile([P, NPAIR * TQ], F16)    # 8 KB/part
        kt_sb = persist.tile([P, KD * T], F16)        # 32 KB/part
        ident = persist.tile([P, P], F16)
        make_identity(nc, ident)
        # ones columns in every (tok-chunk, head) V slot
        onec = persist.tile([P, 1], F16)
        nc.vector.memset(onec[:], 1.0)
        nc.vector.tensor_copy(
            v_sb.rearrange("p (b c) -> p b c", c=VW)[:, :, DH:DH + 1],
            onec.unsqueeze(1).broadcast_to((P, NT * H, 1)))

        # wqkv viewed as [p, ko, col]: one DMA per weight column strip
        wq3 = wqkv.rearrange("(ko p) c -> p ko c", p=P)

        with (
            tc.tile_pool(name="xtp", bufs=1) as xtp,   # spans proj + attention
            tc.tile_pool(name="wp", bufs=1) as wp,
        ):
            xt = xtp.tile([P, KD * T], F16)    # 32 KB/part
            xqt = xtp.tile([P, KD * TQ], F16)  # 8 KB/part

            # ---------- pre-region: A (transpose), C (V), D (QT), B[do=0] ---
            with (
                tc.tile_pool(name="xin", bufs=3) as xinp,
                tc.tile_pool(name="trps", bufs=3, space="PSUM") as trps,
                tc.tile_pool(name="pjps", bufs=5, space="PSUM") as pjps,
            ):
                # A: PE-transpose xq then xb.  8 kd-subtiles share one psum
                # bank; one strided copy scatters them into xt/xqt.
                def transpose_chunk(src_row, dst, dst_off, dst_stride):
                    ps = trps.tile([P, KD * P], F16, tag="tr")
                    for kd in range(KD):
                        nc.tensor.transpose(
                            ps[:, kd * P:(kd + 1) * P],
                            src_row[:, kd * P:(kd + 1) * P], ident[:])
                    nc.vector.tensor_copy(
                        dst.rearrange("p (k c) -> p k c", c=dst_stride)
                           [:, :, dst_off:dst_off + P],
                        ps.rearrange("p (k c) -> p k c", c=P))

                for tci in range(NQ):
                    xin = xinp.tile([P, D], F16, tag="xin")
                    nc.sync.dma_start(xin[:], xq[tci * P:(tci + 1) * P, :])
                    transpose_chunk(xin, xqt, tci * P, TQ)
                for tci in range(NT):
                    xin = xinp.tile([P, D], F16, tag="xin")
                    nc.sync.dma_start(xin[:], xb[tci * P:(tci + 1) * P, :])
                    transpose_chunk(xin, xt, tci * P, T)

                # D: QT (dout chunk do covers heads 2do, 2do+1)
                for do in range(KD):
                    wt = wp.tile([P, KD * P], F16, tag="wk", bufs=2)
                    nc.sync.dma_start(
                        wt.rearrange("p (ko c) -> p ko c", c=P),
                        wq3[:, :, do * P:(do + 1) * P])
                    pq = pjps.tile([P, TQ], F32, tag="pj")
                    for kd in range(KD):
                        nc.tensor.matmul(
                            pq[:], wt[:, kd * P:(kd + 1) * P],
                            xqt[:, kd * TQ:(kd + 1) * TQ],
                            start=(kd == 0), stop=(kd == KD - 1))
                    nc.vector.tensor_copy(qt_sb[:, do * TQ:(do + 1) * TQ],
                                          pq[:])

                # C: V natural, into 65-wide head slots
                for nh in range(2):
                    wvt = wp.tile([P, KD * TQ], F16, tag="wv", bufs=2)
                    nc.sync.dma_start(
                        wvt.rearrange("p (ko c) -> p ko c", c=TQ),
                        wq3[:, :, 2 * D + nh * TQ: 2 * D + (nh + 1) * TQ])
                    for tci in range(NT):
                        pv = pjps.tile([P, TQ], F32, tag="pj")
                        for kd in range(KD):
                            nc.tensor.matmul(
                                pv[:],
                                xt[:, kd * T + tci * P: kd * T + (tci + 1) * P],
                                wvt[:, kd * TQ:(kd + 1) * TQ],
                                start=(kd == 0), stop=(kd == KD - 1))
                        dst = v_sb[:, tci * (H * VW) + nh * 8 * VW:
                                   tci * (H * VW) + (nh + 1) * 8 * VW]
                        nc.vector.tensor_copy(
                            dst.rearrange("p (h c) -> p h c", c=VW)[:, :, 0:DH],
                            pv.rearrange("p (h c) -> p h c", c=DH))

                # B[do=0]: pre-computed so pair 0 can start immediately;
                # kd-outer with 4 open accumulators amortizes LDWEIGHTS.
                wt = wp.tile([P, KD * P], F16, tag="wk", bufs=2)
                nc.sync.dma_start(
                    wt.rearrange("p (ko c) -> p ko c", c=P),
                    wq3[:, :, D: D + P])
                pks = [pjps.tile([P, TQ], F32, tag="pj", name=f"pk0_{_t}")
                       for _t in range(NTB)]
                for kd in range(KD):
                    for tb in range(NTB):
                        nc.tensor.matmul(
                            pks[tb][:], wt[:, kd * P:(kd + 1) * P],
                            xt[:, kd * T + tb * TQ: kd * T + (tb + 1) * TQ],
                            start=(kd == 0), stop=(kd == KD - 1))
                for tb in range(NTB):
                    nc.vector.tensor_copy(
                        kt_sb[:, tb * TQ:(tb + 1) * TQ], pks[tb][:])

            # ---------- region: attention pairs interleaved with B[do] -----
            with (
                tc.tile_pool(name="otp", bufs=1) as otp,
                tc.tile_pool(name="ptp", bufs=4) as ptp,
                tc.tile_pool(name="rcp", bufs=2) as rcp,
                tc.tile_pool(name="rbp", bufs=3) as rbp,
                tc.tile_pool(name="wop", bufs=16) as wop,
            ):
                ot_sb = otp.tile([P, NPAIR * TQ], F16)      # 8 KB/part
                # preload Wo so phase F never waits on DMA
                wot = {}
                for ph in range(NPAIR):
                    for nh in range(2):
                        wot[ph, nh] = wop.tile([P, TQ], F16, tag="wo",
                                               name=f"wo_{ph}_{nh}")
                        nc.sync.dma_start(
                            wot[ph, nh][:],
                            wo[ph * P:(ph + 1) * P, nh * TQ:(nh + 1) * TQ])

                GRP = 2   # 512-wide logits halves per psum tile / exp call
                attn_ps = tc.tile_pool(name="lgps", bufs=2, space="PSUM")
                lgps = attn_ps.__enter__()
                pv_ps = tc.tile_pool(name="pvps", bufs=2, space="PSUM")
                pvps = pv_ps.__enter__()
                b_ps = tc.tile_pool(name="bjps", bufs=2, space="PSUM")
                bjps = b_ps.__enter__()

                def emit_b_chunk(do, tb):
                    # One KT (row chunk do, token block tb) accumulation:
                    # PE filler spread through the ACT-bound attention.
                    # Copy-out goes on ScalarE: ACT has slack and this keeps
                    # the PSUM-bank free off the Vector queue (where it would
                    # sit behind the previous pair's reciprocal and stall the
                    # in-order PE queue).
                    wt = bwt[do]
                    pk = bjps.tile([P, TQ], F32, tag="bk",
                                   name=f"pkb{do}_{tb}")
                    for kd in range(KD):
                        nc.tensor.matmul(
                            pk[:], wt[:, kd * P:(kd + 1) * P],
                            xt[:, kd * T + tb * TQ: kd * T + (tb + 1) * TQ],
                            start=(kd == 0), stop=(kd == KD - 1))
                    nc.scalar.copy(
                        kt_sb[:, do * T + tb * TQ: do * T + (tb + 1) * TQ],
                        pk[:])

                bwt = {}

                def fetch_b_weights(do):
                    bwt[do] = wp.tile([P, KD * P], F16, tag="wk", bufs=2,
                                      name=f"wtb{do}")
                    nc.sync.dma_start(
                        bwt[do].rearrange("p (ko c) -> p ko c", c=P),
                        wq3[:, :, D + do * P: D + (do + 1) * P])

                for p in range(NPAIR):
                    if p + 1 < NPAIR:
                        fetch_b_weights(p + 1)
                    kt = kt_sb[:, p * T:(p + 1) * T]
                    qh = (qt_sb[0:DH, p * TQ:(p + 1) * TQ],
                          qt_sb[DH:P, p * TQ:(p + 1) * TQ])
                    pva = pvps.tile([VW, TQ], F32, tag="pv")
                    pvb = pvps.tile([VW, TQ], F32, tag="pv")
                    halves = [(kc, hh) for kc in range(NT) for hh in (0, 1)]
                    groups = [halves[i:i + GRP]
                              for i in range(0, len(halves), GRP)]
                    loc = {}
                    emitted = set()

                    def emit_pv_ready(done_through, p=p, pva=pva, pvb=pvb,
                                      loc=loc, emitted=emitted):
                        for kc in range(NT):
                            if kc in emitted:
                                continue
                            if ((kc, 0) not in done_through
                                    or (kc, 1) not in done_through):
                                return
                            emitted.add(kc)
                            for hh, pv_ in ((0, pva), (1, pvb)):
                                h = 2 * p + hh
                                va = v_sb[:, kc * (H * VW) + h * VW:
                                          kc * (H * VW) + h * VW + VW]
                                pt_, j = loc[kc, hh]
                                nc.tensor.matmul(pv_[:], va,
                                                 pt_[:, j * TQ:(j + 1) * TQ],
                                                 start=(kc == 0),
                                                 stop=(kc == NT - 1))

                    done = set()
                    prev_done = set()
                    for gi, grp in enumerate(groups):
                        # spread next pair's KT chunks through this pair
                        if p + 1 < NPAIR and gi % 4 == 2:
                            emit_b_chunk(p + 1, gi // 4)
                        emit_pv_ready(prev_done)
                        n = len(grp)
                        lg = lgps.tile([P, GRP * TQ], F32, tag="lg")
                        for j, (kc, hh) in enumerate(grp):
                            nc.tensor.matmul(
                                lg[:, j * TQ:(j + 1) * TQ],
                                kt[hh * DH:(hh + 1) * DH,
                                   kc * P:(kc + 1) * P],
                                qh[hh], start=True, stop=True)
                        pt_ = ptp.tile([P, GRP * TQ], F16, tag="pt")
                        nc.scalar.activation(pt_[:, 0:n * TQ],
                                             lg[:, 0:n * TQ],
                                             EXPF, scale=0.125)
                        for j, half in enumerate(grp):
                            loc[half] = (pt_, j)
                        prev_done = set(done)
                        done.update(grp)
                    emit_pv_ready(done)

                    # normalize: outT_h[dh, q] *= 1 / s[q].  Both psum copies
                    # are emitted FIRST so the PV banks free for the next pair
                    # before the reciprocals run.
                    pvs_t = {}
                    for hi, pv_ in ((0, pva), (1, pvb)):
                        pvs = rcp.tile([VW, TQ], F32, tag="pvs")
                        nc.vector.tensor_copy(pvs[:], pv_[:])
                        pvs_t[hi] = pvs
                    for hi in (0, 1):
                        pvs = pvs_t[hi]
                        rc = rcp.tile([P, TQ], F32, tag="rc")
                        nc.vector.reciprocal(rc[DH:DH + 1, :],
                                             pvs[DH:DH + 1, :])
                        rb = rbp.tile([P, TQ], F32, tag="rb")
                        nc.sync.dma_start(
                            rb[0:DH, :],
                            rc[DH:DH + 1, :].unsqueeze(1)
                              .broadcast_to((1, DH, TQ)))
                        if hi == 0:
                            nc.vector.tensor_mul(
                                ot_sb[0:DH, p * TQ:(p + 1) * TQ],
                                pvs[0:DH, :], rb[0:DH, :])
                        else:
                            # head b lands at partitions 64-127 of ot_sb;
                            # DVE cannot shift partitions, so normalize into
                            # a staging tile then DMA-shift.
                            sh = rbp.tile([P, TQ], F16, tag="sh")
                            nc.vector.tensor_mul(
                                sh[0:DH, :], pvs[0:DH, :], rb[0:DH, :])
                            nc.sync.dma_start(
                                ot_sb[DH:P, p * TQ:(p + 1) * TQ],
                                sh[0:DH, :])
                b_ps.__exit__(None, None, None)
                pv_ps.__exit__(None, None, None)
                attn_ps.__exit__(None, None, None)

                # F: y = outT^T @ Wo (ph-outer reuses each stationary twice)
                with tc.tile_pool(name="fps", bufs=4, space="PSUM") as fps:
                  for qc in range(NQ):
                    pys = [fps.tile([P, TQ], F32, tag="f", name=f"py{qc}_{_n}")
                           for _n in range(2)]
                    for ph in range(NPAIR):
                        for nh in range(2):
                            nc.tensor.matmul(
                                pys[nh][:],
                                ot_sb[:, ph * TQ + qc * P:
                                      ph * TQ + (qc + 1) * P],
                                wot[ph, nh][:],
                                start=(ph == 0), stop=(ph == NPAIR - 1))
                    for nh in range(2):
                        ys = rbp.tile([P, TQ], F32, tag="rb")
                        nc.scalar.copy(ys[:], pys[nh][:])
                        nc.sync.dma_start(
                            y[qc * P:(qc + 1) * P, nh * TQ:(nh + 1) * TQ],
                            ys[:])
    nc.compile()
    return nc


_NC_CACHE = None


def _get_nc():
    global _NC_CACHE
    if _NC_CACHE is None:
        _NC_CACHE = build_nc()
    return _NC_CACHE


def _shard_inputs(x, Wqkv, Wo):
    x16 = np.asarray(x, dtype=np.float32).astype(np.float16)
    w16 = np.ascontiguousarray(
        np.asarray(Wqkv, dtype=np.float32).astype(np.float16))
    wo16 = np.ascontiguousarray(
        np.asarray(Wo, dtype=np.float32).astype(np.float16))
    in_maps = []
    for c in range(N_CORES):
        b, tb = c // NTB, c % NTB
        in_maps.append({
            "xb": np.ascontiguousarray(x16[b]),
            "xq": np.ascontiguousarray(x16[b, tb * TQ:(tb + 1) * TQ, :]),
            "wqkv": w16,
            "wo": wo16,
        })
    return in_maps


def kernel(x, Wqkv, Wo):
    from concourse.bass_utils import run_bass_kernel_spmd

    nc = _get_nc()
    in_maps = _shard_inputs(x, Wqkv, Wo)
    res = run_bass_kernel_spmd(nc, in_maps, core_ids=list(range(N_CORES)))
    out = np.empty((B, T, D), dtype=np.float32)
    for c in range(N_CORES):
        b, tb = c // NTB, c % NTB
        out[b, tb * TQ:(tb + 1) * TQ, :] = res.results[c]["y"]
    return out
